# revision 13
# baseline (speedup 1.0000x reference)
"""Multi-head causal attention on 8 Trainium2 cores.

Sharding: core = (batch b in 0..3, head-group g in 0..1). Each core computes
Q/K/V projections for its 8 heads of its batch, causal attention, and a
partial output projection (Wo row-split); host sums the two partials per
batch and transposes back.

v3 design notes (on top of v2's bf16 + SBUF-resident Q/K/V/Wo):
  - TRN2 matmul throughput is capped by the fp32 PSUM drain at 1 output
    column/cycle, so output-column count is the currency.  The softmax
    denominator (Z) stream is cut ~2.6x by pre-summing the probability
    tiles of each chunk's full k-tiles on the (idle) Vector engine in
    fp32, so one ones-matmul per chunk covers all full tiles.
  - Half-0's output projection is interleaved into half-1's attention
    pair loop (one 4-matmul step per pair) so PE does outproj work while
    ACT runs exp, instead of serializing an ACT-idle outproj block after
    ACT-bound attention.
  - A short dummy-matmul warmup right after the first weight tile lands
    warms the PE HAM clock gate during the x-load window, and the first
    two Q heads are computed jointly k-major (8 psum banks) so the PE
    keeps pace with the x k-row DMAs during startup.
"""

import numpy as np
import ml_dtypes

import concourse.bacc as bacc
import concourse.mybir as mybir
import concourse.tile as tile
from concourse.bass_utils import run_bass_kernel_spmd

B, T, D = 4, 2048, 2048
NH, HD = 16, 128
G = 8                       # heads per core
GD = G * HD                 # 1024, group channel width
P = 128
QC = 512                    # q-chunk width (PSUM bank width in fp32)
NKT = T // P                # 16 k-tiles over the sequence
NDK = D // P                # 16 k-tiles over d_in
NQC = T // QC               # 4 q-chunks
SCALE = 1.0 / float(np.sqrt(HD))

F32 = mybir.dt.float32
F32R = mybir.dt.float32r
BF16 = mybir.dt.bfloat16
NPBF = ml_dtypes.bfloat16


def build_kernel():
    nc = bacc.Bacc("TRN2", target_bir_lowering=False, debug=False, num_devices=8,
                   dynamic_dma_scratch_size=2048)

    xT = nc.dram_tensor("xT", [D, T], BF16, kind="ExternalInput")
    # pre-tiled on host: wq/wk [head, p, ko, d], wv [dchunk, p, ko, c]
    wqT = nc.dram_tensor("wqT", [G, P, NDK, HD], BF16, kind="ExternalInput")
    wkT = nc.dram_tensor("wkT", [G, P, NDK, HD], BF16, kind="ExternalInput")
    wvT = nc.dram_tensor("wvT", [2, P, NDK, QC], BF16, kind="ExternalInput")
    woT = nc.dram_tensor("woT", [GD, D], BF16, kind="ExternalInput")
    # maskc[:, 0:128] = causal 0/1 triangle (1 iff col >= row), [:,128:256]=1
    maskc = nc.dram_tensor("maskc", [P, 2 * P], BF16, kind="ExternalInput")
    outT = nc.dram_tensor("outT", [D, T], F32, kind="ExternalOutput")

    xT_t = xT.rearrange("(ko p) t -> p ko t", p=P)
    woT_t = woT.rearrange("(co p) n -> p co n", p=P)
    outT_t = outT.rearrange("(no p) t -> p no t", p=P)

    with tile.TileContext(nc) as tc:
        # Whole-kernel SBUF residents: Q^T [d, chunk, head, q], K^T [d, head, t],
        # V [t%128, t-tile, head, hd].  4 MB each in bf16.
        with tc.tile_pool(name="res", bufs=1) as res:
            q_res = res.tile([P, NQC, G, QC], BF16)
            k_res = res.tile([P, G, T], BF16)
            v_res = res.tile([P, NKT, G, HD], BF16)
            maskc_sb = res.tile([P, 2 * P], BF16)

            # ---------------- Phase A: projections ----------------
            with (
                tc.tile_pool(name="xpool", bufs=1) as xpool,
                tc.tile_pool(name="wpool", bufs=3) as wpool,
                tc.tile_pool(name="wvpool", bufs=1) as wvpool,
                tc.tile_pool(name="psA", bufs=8, space="PSUM") as psA,
            ):
                # First two weight tiles go on the scalar DMA queue so they
                # don't serialize ahead of the x k-rows; the x rows alternate
                # between the sync and gpsimd queues for parallel rings.
                w01 = []
                for h in range(2):
                    w_sb = wpool.tile([P, NDK, HD], BF16, tag="w")
                    nc.scalar.dma_start(w_sb, wqT[h])
                    w01.append(w_sb)
                nc.scalar.dma_start(maskc_sb, maskc[:])

                xt_sb = xpool.tile([P, NDK, T], BF16)    # 8 MB, resident
                for k in range(NDK):
                    eng = nc.sync if k % 2 == 0 else nc.gpsimd
                    eng.dma_start(xt_sb[:, k], xT_t[:, k])

                # HAM warmup: dummy matmuls on the first weight tile keep PE
                # busy (and un-throttled) while the x k-rows stream in.
                warm_ps = psA.tile([P, QC], F32, tag="ps")
                wmov = w01[0].rearrange("p a b -> p (a b)")
                for i in range(16):
                    nc.tensor.matmul(warm_ps, w01[0][:, i % NDK],
                                     wmov[:, 0:QC], start=True, stop=True)

                # Q^T and K^T: out[d(128), t] = sum_k W^T[k, d] x^T[k, t]
                # k-outer: 4 chunk accumulators per head stay live.  The
                # first two Q heads run jointly (8 banks) to keep pace with
                # the x DMA during startup.
                sched = [(0, (0, 1))] + [(0, (h,)) for h in range(2, G)]
                sched += [(1, (h,)) for h in range(G)]
                for wt, hs in sched:
                    w_sbs = {}
                    for h in hs:
                        if wt == 0 and h < 2:
                            w_sbs[h] = w01[h]
                        else:
                            w_sb = wpool.tile([P, NDK, HD], BF16, tag="w")
                            nc.sync.dma_start(w_sb, (wqT, wkT)[wt][h])
                            w_sbs[h] = w_sb
                    ps_c = {h: [psA.tile([P, QC], F32, tag="ps",
                                         name=f"ps_{wt}_{h}_{c}")
                                for c in range(NQC)] for h in hs}
                    for k in range(NDK):
                        for h in hs:
                            for c in range(NQC):
                                nc.tensor.matmul(
                                    ps_c[h][c],
                                    w_sbs[h][:, k],
                                    xt_sb[:, k, c * QC:(c + 1) * QC],
                                    start=(k == 0),
                                    stop=(k == NDK - 1),
                                )
                    for h in hs:
                        for c in range(NQC):
                            if wt == 0:
                                nc.vector.tensor_copy(q_res[:, c, h],
                                                      ps_c[h][c])
                            else:
                                nc.scalar.copy(
                                    k_res[:, h, c * QC:(c + 1) * QC],
                                    ps_c[h][c])

                # V: out[t(128), c(512)] = sum_k x^T[k, t] wvT[k, c]
                # x^T tile is the stationary, shared by the two d-chunks.
                wv_sbs = []
                for dc in range(2):
                    wv_sb = wvpool.tile([P, NDK, QC], BF16, tag=f"wv{dc}")
                    nc.sync.dma_start(wv_sb, wvT[dc])
                    wv_sbs.append(wv_sb)
                for ts_ in range(NKT):
                    ps_d = [psA.tile([P, QC], F32, tag="ps",
                                     name=f"ps_v_{ts_}_{dc}")
                            for dc in range(2)]
                    for k in range(NDK):
                        for dc in range(2):
                            nc.tensor.matmul(
                                ps_d[dc],
                                xt_sb[:, k, ts_ * P:(ts_ + 1) * P],
                                wv_sbs[dc][:, k],
                                start=(k == 0),
                                stop=(k == NDK - 1),
                            )
                    for dc in range(2):
                        nc.vector.tensor_copy(
                            v_res[:, ts_, 4 * dc:4 * dc + 4, :].rearrange(
                                "p g c -> p (g c)"),
                            ps_d[dc],
                        )

            # ---------------- Phase B: attention + output projection ----------------
            with (
                tc.tile_pool(name="wopool", bufs=1) as wopool,
                tc.tile_pool(name="const", bufs=1) as constp,
                tc.tile_pool(name="ppool", bufs=4) as ppool,
                tc.tile_pool(name="accp", bufs=2) as accp,
                tc.tile_pool(name="cpool", bufs=2) as cpool,
                tc.tile_pool(name="zpool", bufs=2) as zpool,
                tc.tile_pool(name="opool", bufs=3) as opool,
                tc.tile_pool(name="psS", bufs=2, space="PSUM") as psS,
                tc.tile_pool(name="psZ", bufs=1, space="PSUM") as psZ,
                tc.tile_pool(name="psC", bufs=2, space="PSUM") as psC,
                tc.tile_pool(name="psO", bufs=1, space="PSUM") as psO,
            ):
                tri01 = maskc_sb[:, 0:P]
                ones_bf = maskc_sb[:, P:2 * P]
                ones_fr = constp.tile([P, P], F32R)
                nc.vector.memset(ones_fr.bitcast(F32), 1.0)
                wo_sb = wopool.tile([P, G, D], BF16)      # 4 MB, resident

                class OutProj:
                    """Half-a-psum-group (4 matmuls) per drain_step call, so
                    outproj work interleaves finely between attention pairs.
                    maybe_drain spreads the steps evenly over n_slots calls."""

                    def __init__(self, ctx2, chunks, n_slots):
                        self.items = [(ci, c, nt)
                                      for ci, c in enumerate(chunks)
                                      for nt in range(NDK)]
                        self.ctx2 = ctx2
                        self.idx = 0
                        self.step = 0
                        self.o_ps = None
                        self.n_steps = 2 * len(self.items)
                        self.n_slots = max(1, n_slots)
                        self.calls = 0
                        self.steps_done = 0

                    def done(self):
                        return self.idx >= len(self.items)

                    def maybe_drain(self):
                        self.calls += 1
                        want = self.calls * self.n_steps // self.n_slots
                        while self.steps_done < want and not self.done():
                            self.drain_step()
                            self.steps_done += 1

                    def drain_step(self):
                        if self.done():
                            return
                        ci, c, nt = self.items[self.idx]
                        if self.step == 0:
                            self.o_ps = psO.tile([P, QC], F32, tag="o",
                                                 name=f"o_ps_{c}_{nt}")
                        for hh in range(4 * self.step, 4 * self.step + 4):
                            nc.tensor.matmul(
                                self.o_ps,
                                wo_sb[:, hh, nt * P:(nt + 1) * P],
                                self.ctx2[:, ci, hh],
                                start=(hh == 0),
                                stop=(hh == G - 1),
                            )
                        self.step += 1
                        if self.step == 2:
                            o_sb = opool.tile([P, QC], F32, tag="o_sb")
                            nc.scalar.copy(o_sb, self.o_ps)
                            nc.sync.dma_start(
                                outT_t[:, nt, c * QC:(c + 1) * QC], o_sb)
                            self.idx += 1
                            self.step = 0
                            self.o_ps = None

                # one-pair software pipeline: PV (and chunk finalization) of
                # pair i issue after S+exp of pair i+1.
                pend = [None]

                def flush():
                    if pend[0] is not None:
                        fn = pend[0]
                        pend[0] = None
                        fn()

                op_q = [None]    # OutProj of the previous half, drained here

                for half in range(NQC // 2):
                    chunks = (2 * half, 2 * half + 1)
                    ctx2_sb = cpool.tile([P, 2, G, QC], BF16, tag="ctx2")

                    for h in range(G):
                        for ci, c in enumerate(chunks):
                            n_kt = 4 * (c + 1)
                            ctx_ps = psC.tile([P, QC], F32, tag="ctx")

                            # full pairs first, the two diagonal pairs last
                            kts = [kt for kt in range(n_kt) if kt < 4 * c]
                            kts += [kt for kt in range(n_kt) if kt >= 4 * c]
                            pairs = [(kts[i], kts[i + 1])
                                     for i in range(0, n_kt, 2)]
                            acc = [None]     # fp32 running sum of full P tiles
                            diag_zs = []     # (s0, p2, off) for diag k-tiles

                            for pi, (ka, kb) in enumerate(pairs):
                                s0a = max(0, (ka - 4 * c)) * P
                                s0b = max(0, (kb - 4 * c)) * P
                                s2 = psS.tile([P, 2 * QC], F32, tag="s2")
                                p2 = ppool.tile([P, 2 * QC], BF16, tag="p2")
                                wb = 0 if s0b <= P else s0b
                                nc.tensor.matmul(
                                    s2[:, s0a:QC],
                                    k_res[:, h, ka * P:(ka + 1) * P],
                                    q_res[:, c, h, s0a:QC],
                                    start=True, stop=True,
                                )
                                nc.tensor.matmul(
                                    s2[:, QC + wb:2 * QC],
                                    k_res[:, h, kb * P:(kb + 1) * P],
                                    q_res[:, c, h, wb:QC],
                                    start=True, stop=True,
                                )
                                if wb == 0:
                                    nc.scalar.activation(
                                        p2[:, s0a:2 * QC], s2[:, s0a:2 * QC],
                                        mybir.ActivationFunctionType.Exp,
                                        scale=SCALE,
                                    )
                                else:
                                    nc.scalar.activation(
                                        p2[:, s0a:QC], s2[:, s0a:QC],
                                        mybir.ActivationFunctionType.Exp,
                                        scale=SCALE,
                                    )
                                    nc.scalar.activation(
                                        p2[:, QC + s0b:2 * QC],
                                        s2[:, QC + s0b:2 * QC],
                                        mybir.ActivationFunctionType.Exp,
                                        scale=SCALE,
                                    )
                                if ka >= 4 * c:
                                    # diagonal pair: mask + defer Z to the
                                    # chunk-end sliced matmuls
                                    for idx, kt, s0 in ((0, ka, s0a),
                                                        (1, kb, s0b)):
                                        j = kt - 4 * c
                                        r0 = idx * QC + j * P
                                        nc.vector.tensor_mul(
                                            p2[:, r0:r0 + P],
                                            p2[:, r0:r0 + P],
                                            tri01,
                                        )
                                        diag_zs.append((s0, p2, idx * QC))
                                else:
                                    # full pair: fold P into the fp32 running
                                    # sum on DVE (exact; one Z matmul later)
                                    if acc[0] is None:
                                        a = accp.tile([P, QC], F32R, tag="acc")
                                        nc.vector.tensor_add(
                                            a, p2[:, 0:QC], p2[:, QC:2 * QC])
                                        acc[0] = a
                                    else:
                                        nc.vector.tensor_add(
                                            acc[0], acc[0], p2[:, 0:QC])
                                        nc.vector.tensor_add(
                                            acc[0], acc[0], p2[:, QC:2 * QC])

                                flush()
                                if op_q[0] is not None:
                                    op_q[0].maybe_drain()

                                def mk(pi, ka, kb, s0a, s0b, p2, ctx_ps, h,
                                       ci, c, n_kt, acc, diag_zs, z_dst):
                                    def go():
                                        for idx, kt, s0 in (
                                                (0, ka, s0a), (1, kb, s0b)):
                                            ki = 2 * pi + idx
                                            nc.tensor.matmul(
                                                ctx_ps[:, s0:QC],
                                                v_res[:, kt, h],
                                                p2[:, idx * QC + s0:
                                                    idx * QC + QC],
                                                start=(ki == 0),
                                                stop=(ki == n_kt - 1),
                                            )
                                        if 2 * pi + 2 == n_kt:
                                            z_ps = psZ.tile([P, QC], F32,
                                                            tag="z")
                                            nops = (1 if acc[0] is not None
                                                    else 0) + len(diag_zs)
                                            i = 0
                                            if acc[0] is not None:
                                                nc.tensor.matmul(
                                                    z_ps, ones_fr, acc[0],
                                                    start=True,
                                                    stop=(nops == 1),
                                                )
                                                i = 1
                                            for s0, pz, off in diag_zs:
                                                nc.tensor.matmul(
                                                    z_ps[:, s0:QC], ones_bf,
                                                    pz[:, off + s0:off + QC],
                                                    start=(i == 0),
                                                    stop=(i == nops - 1),
                                                )
                                                i += 1
                                            iz = zpool.tile([P, QC], F32,
                                                            tag="iz")
                                            nc.vector.reciprocal_approx_fast(
                                                iz, z_ps)
                                            nc.vector.tensor_mul(
                                                z_dst, ctx_ps, iz)
                                    return go

                                pend[0] = mk(pi, ka, kb, s0a, s0b, p2, ctx_ps,
                                             h, ci, c, n_kt, acc, diag_zs,
                                             ctx2_sb[:, ci, h])

                        if half == 0:
                            nc.sync.dma_start(wo_sb[:, h], woT_t[:, h])

                    flush()
                    while op_q[0] is not None and not op_q[0].done():
                        op_q[0].drain_step()
                    if half == 0:
                        # chunks 0/1's outproj interleaves into half-1's
                        # attention; half-1 has 8*(6+8) = 112 pairs.
                        op_q[0] = OutProj(ctx2_sb, chunks, n_slots=112)
                    else:
                        # nothing left to overlap with: run the last half's
                        # outproj as a straight pipelined loop (2-bank psum
                        # units, copies on the otherwise-idle ACT engine).
                        for ci, c in enumerate(chunks):
                            q_sl = slice(c * QC, (c + 1) * QC)
                            for nt2 in range(NDK // 2):
                                o2 = psS.tile([P, 2 * QC], F32, tag="s2")
                                for hn in range(2):
                                    nt = 2 * nt2 + hn
                                    for hh in range(G):
                                        nc.tensor.matmul(
                                            o2[:, hn * QC:(hn + 1) * QC],
                                            wo_sb[:, hh, nt * P:(nt + 1) * P],
                                            ctx2_sb[:, ci, hh],
                                            start=(hh == 0),
                                            stop=(hh == G - 1),
                                        )
                                for hn in range(2):
                                    nt = 2 * nt2 + hn
                                    o_sb = opool.tile([P, QC], F32,
                                                      tag="o_sb")
                                    nc.scalar.copy(
                                        o_sb, o2[:, hn * QC:(hn + 1) * QC])
                                    nc.sync.dma_start(
                                        outT_t[:, nt, q_sl], o_sb)

    nc.finalize()
    return nc


_NC = None


def _get_nc():
    global _NC
    if _NC is None:
        _NC = build_kernel()
    return _NC


def _make_maskc():
    m = np.zeros((P, 2 * P), dtype=NPBF)
    i = np.arange(P)[:, None]
    col = np.arange(P)[None, :]
    m[:, 0:P] = (col >= i).astype(NPBF)   # keep iff col >= row
    m[:, P:2 * P] = NPBF(1.0)
    return m


def kernel(x, Wq, Wk, Wv, Wo, _trace=False, _trace_kwargs=None):
    x = np.asarray(x, dtype=np.float32)
    Wq = np.asarray(Wq, dtype=np.float32)
    Wk = np.asarray(Wk, dtype=np.float32)
    Wv = np.asarray(Wv, dtype=np.float32)
    Wo = np.asarray(Wo, dtype=np.float32)

    nc = _get_nc()
    maskc = _make_maskc()

    # [d_out, d_in] -> [h, p, ko, dd] tiles per head-group chunk of 8 heads
    def tile_qk(W, g):
        wt = W.T[:, g * GD:(g + 1) * GD]              # [D, GD]
        return np.ascontiguousarray(
            wt.reshape(NDK, P, G, HD).transpose(2, 1, 0, 3)).astype(NPBF)

    def tile_v(W, g):
        wt = W.T[:, g * GD:(g + 1) * GD]              # [D, GD]
        return np.ascontiguousarray(
            wt.reshape(NDK, P, 2, QC).transpose(2, 1, 0, 3)).astype(NPBF)

    woT = np.ascontiguousarray(Wo.T)

    per_g = {}
    for g in range(2):
        gs = slice(g * GD, (g + 1) * GD)
        per_g[g] = {
            "wqT": tile_qk(Wq, g),
            "wkT": tile_qk(Wk, g),
            "wvT": tile_v(Wv, g),
            "woT": np.ascontiguousarray(woT[gs, :]).astype(NPBF),
        }

    in_maps = []
    for core in range(8):
        b, g = divmod(core, 2)
        in_maps.append({
            "xT": np.ascontiguousarray(x[b].T).astype(NPBF),
            "maskc": maskc,
            **per_g[g],
        })

    kwargs = {}
    if _trace:
        kwargs.update(trace=True, **(_trace_kwargs or {}))
    res = run_bass_kernel_spmd(nc, in_maps, core_ids=list(range(8)), **kwargs)

    out = np.empty((B, T, D), dtype=np.float32)
    for b in range(B):
        acc = res.results[2 * b]["outT"] + res.results[2 * b + 1]["outT"]
        out[b] = acc.T
    if _trace:
        return out, res
    return out
